# revision 18
# baseline (speedup 1.0000x reference)
"""Multi-head causal attention (B=2, S=2048, D=1024, H=16) on 8 trn2 cores.

Sharding: core c -> (batch b = c//4, head-group g = c%4, 4 heads each).
Data-parallel over B, tensor-parallel over heads. Each core computes a
partial output projection [S, D] in bf16; the host sums the 4 partials
per batch (in f32) and adds b_out.

Device kernel per core (matmuls in bf16, accumulation in f32 PSUM):
  A) qkT[f=512, s=2048] = (x @ Wqk)^T and v[s, f=256] = x @ Wv (+biases,
     folded in as K=1 rank-1 f32r matmuls into PSUM).
     qkT feature layout: [K(h0)|K(h1)] [Q(h0)|Q(h1)] [K(h2)|K(h3)] [Q(h2)|Q(h3)]
     PSUM->SBUF(bf16) moves: qkT on the Act engine, v on DVE.
  B) per head h, per 512-wide query block qmb: causal flash attention in
     the scores-TRANSPOSED layout: sT[k,q] = K @ Q^T so that attn@V is
     lhsT=v_blk[s,hd+1] (ones col appended -> softmax denominators in
     row 64 of PSUM), rhs=expT[k,q]. No on-chip transposes anywhere.
     Causal mask applied POST-exp: DVE multiplies the diagonal 128x128
     blocks of ex (bf16, SBUF -> 4x DVE mode) by a 0/1 triangle. This
     keeps the PE free of mask matmuls/Ldweights.
     The sc->exp->av chains are software-pipelined (av emission lags by
     `lag` links) and rotate through a deep PSUM pool so cross-engine
     semaphore wake latency is hidden.
  C) out_partial[s, 1024] = values^T.T @ W_out; PSUM->SBUF copies on the
     (otherwise idle) Pool engine, DMA'd to DRAM as bf16.
"""
import json
import math
import os

import numpy as np
import ml_dtypes

import concourse.bass as bass
import concourse.mybir as mybir
import concourse.tile as tile
from concourse import bacc
from concourse.bass_utils import run_bass_kernel_spmd

N_CORES = 8
B, S, D = 2, 2048, 1024
H = 16                    # total heads
HL = 4                    # heads per core
HD = 64                   # head dim
FQK = 2 * HL * HD         # 512 local q+k features
FV = HL * HD              # 256 local v features
SCALE = 1.0 / math.sqrt(HD)

QMB = 512                 # query macro-block
KB = 128                  # key block
N_QMB = S // QMB          # 4
N_KB = S // KB            # 16

F32 = mybir.dt.float32
F32R = mybir.dt.float32r
BF16 = mybir.dt.bfloat16


def build_kernel(repeat: int = 1, stages: str = "ABC",
                 bmode: str = "full", pairw: int = 2, wave: int = 2,
                 sc_bufs: int = 3, av_bufs: int = 2, exp_bufs: int = 8,
                 lag: int = 2,
                 fill_first: bool = False,
                 qk_copy_eng: str = "scalar",
                 ob_eng: str = "vector",
                 mask_eng: str = "vector",
                 bdt_name: str = "bf16",
                 staggered: bool = False):
    assert sc_bufs * pairw + av_bufs <= 8
    W = 512 * pairw
    # internal dtype for qkT / v_aug / ex / valuesT (and wo to match C MMs):
    # bf16 halves SBUF + full-rate short MMs, but legalization splits every
    # 2-byte matmul into Ldweights+Matmult (extra PE.SEQ issue slots);
    # f32r self-loads (no Ldweights) at 1 cyc/row for >=256-wide MMs.
    bdt = BF16 if bdt_name == "bf16" else F32R
    nc = bacc.Bacc(
        "TRN2", target_bir_lowering=False, debug=False, num_devices=N_CORES
    )
    xT = nc.dram_tensor("xT", [D, S], BF16, kind="ExternalInput")
    wqk = nc.dram_tensor("wqk", [D, FQK], BF16, kind="ExternalInput")
    wv = nc.dram_tensor("wv", [D, FV], BF16, kind="ExternalInput")
    wo = nc.dram_tensor("wo", [FV, D], bdt, kind="ExternalInput")
    bqk = nc.dram_tensor("bqk", [FQK], F32R, kind="ExternalInput")
    bv = nc.dram_tensor("bv", [FV], F32R, kind="ExternalInput")
    out = nc.dram_tensor("out", [S, D], BF16, kind="ExternalOutput")

    KT = D // 128  # 8 contraction tiles over D

    with tile.TileContext(nc) as tc:
        dma = nc.sync  # HWDGE: spreads transfers over HW queues
        with (
            tc.tile_pool(name="const", bufs=1) as const,
            tc.tile_pool(name="xt", bufs=1) as xtp,
            tc.tile_pool(name="big", bufs=1) as big,
            tc.tile_pool(name="exp", bufs=exp_bufs) as expp,
            tc.tile_pool(name="small", bufs=4) as small,
            tc.tile_pool(name="ob", bufs=3) as obp,
            tc.tile_pool(name="ps_sc", bufs=sc_bufs, space="PSUM") as ps_sc,
            tc.tile_pool(name="ps_av", bufs=av_bufs, space="PSUM") as ps_av,
        ):
            # ---- constants ----
            wqk_sb = const.tile([128, KT, FQK], BF16)
            wv_sb = const.tile([128, KT, FV], BF16)
            wo_sb = const.tile([128, FV // 128, D], bdt)
            dma.dma_start(
                out=wqk_sb, in_=wqk.rearrange("(kt p) f -> p kt f", p=128)
            )
            dma.dma_start(
                out=wv_sb, in_=wv.rearrange("(kt p) f -> p kt f", p=128)
            )
            dma.dma_start(
                out=wo_sb, in_=wo.rearrange("(dt p) f -> p dt f", p=128)
            )
            # qk bias laid out per-partition [feature%128, ft] so the Act
            # engine can fold it into the PSUM->SBUF copy (Identity+bias);
            # v bias stays a rank-1 matmul (its features are on the free dim).
            bqk_pb = const.tile([128, FQK // 128], F32R)
            dma.dma_start(
                out=bqk_pb, in_=bqk.rearrange("(ft p) -> p ft", p=128)
            )
            bv_sb = const.tile([1, FV], F32R)
            dma.dma_start(out=bv_sb, in_=bv.rearrange("(o f) -> o f", o=1))
            ones_f32 = const.tile([1, QMB], F32)
            nc.vector.memset(ones_f32, 1.0)
            ones_row = const.tile([1, QMB], F32R)
            nc.vector.tensor_copy(ones_row, ones_f32)
            # multiplicative causal mask for the diagonal 128x128 block:
            # tri01[k, q] = 1 if k <= q else 0  (bf16, for 4x DVE mode)
            tri_f32 = const.tile([128, 128], F32)
            nc.gpsimd.memset(tri_f32, 1.0)
            nc.gpsimd.affine_select(
                out=tri_f32,
                in_=tri_f32,
                compare_op=mybir.AluOpType.is_ge,
                fill=0.0,
                base=0,
                pattern=[[1, 128]],
                channel_multiplier=-1,
            )
            tri01 = const.tile([128, 128], bdt)
            nc.vector.tensor_copy(tri01, tri_f32)

            # ---- persistent intermediates ----
            qkT = big.tile([128, 4, S], bdt)            # 4 f-tiles x S
            v_aug = big.tile([128, N_KB, HL, HD + 1], bdt)
            valuesT = big.tile([128, FV // 128, S], bdt)
            vone_f32 = const.tile([128, N_KB * HL], F32)
            nc.vector.memset(vone_f32, 1.0)
            nc.vector.tensor_copy(
                v_aug[:, :, :, HD:HD + 1],
                vone_f32.rearrange("p (kb h o) -> p kb h o", h=HL, o=1),
            )

            # NOTE: GpSimd (Pool) cannot access PSUM on TRN2 (BIR verifier
            # rejects it), so PSUM->SBUF moves must use DVE or Act.
            if ob_eng == "vector":
                ob_copies = (nc.vector.tensor_copy, nc.vector.tensor_copy)
            elif ob_eng == "scalar":
                ob_copies = (nc.scalar.copy, nc.scalar.copy)
            else:  # "split": one half DVE, one half Act
                ob_copies = (nc.vector.tensor_copy, nc.scalar.copy)
            mask_mul = (nc.vector.tensor_mul if mask_eng == "vector"
                        else nc.gpsimd.tensor_mul)

            def body(_it):
                # ======== stage A: qkT and v_aug, in two column halves ====
                def load_xts(half):
                    s0 = half * (S // 2)
                    xts = []
                    for kt in range(KT):
                        xt_t = xtp.tile([128, S // 2], BF16, tag=f"xt{kt}")
                        dma.dma_start(
                            out=xt_t,
                            in_=xT[kt * 128:(kt + 1) * 128, s0:s0 + S // 2],
                        )
                        xts.append(xt_t)
                    return xts

                def make_qk_item(xts, half, ft, nt):
                    s0 = half * (S // 2)

                    def emit():
                        c0 = nt * 512
                        ps = ps_sc.tile([128, W], F32, tag="sc")
                        for kt in range(KT):
                            nc.tensor.matmul(
                                ps[:, 0:512],
                                wqk_sb[:, kt, ft * 128:(ft + 1) * 128],
                                xts[kt][:, c0:c0 + 512],
                                start=(kt == 0),
                                stop=(kt == KT - 1),
                            )
                        if qk_copy_eng == "scalar":
                            nc.scalar.activation(
                                out=qkT[:, ft, s0 + c0:s0 + c0 + 512],
                                in_=ps[:, 0:512],
                                func=mybir.ActivationFunctionType.Identity,
                                bias=bqk_pb[:, ft:ft + 1],
                            )
                        else:
                            nc.vector.tensor_scalar_add(
                                qkT[:, ft, s0 + c0:s0 + c0 + 512],
                                ps[:, 0:512],
                                bqk_pb.bitcast(F32)[:, ft:ft + 1],
                            )
                    return emit

                def make_v_item(xts, half, stp):
                    def emit():
                        psv = ps_sc.tile([128, 512], F32, tag="sc")
                        for sub in range(2):
                            sti = stp * 2 + sub
                            c0 = sub * FV
                            for kt in range(KT):
                                nc.tensor.matmul(
                                    psv[:, c0:c0 + FV],
                                    xts[kt][:, sti * 128:(sti + 1) * 128],
                                    wv_sb[:, kt, :],
                                    start=(kt == 0),
                                    stop=False,
                                )
                            nc.tensor.matmul(
                                psv[:, c0:c0 + FV],
                                ones_row[0:1, 0:128],
                                bv_sb,
                                start=False,
                                stop=True,
                            )
                        st0 = half * 8 + stp * 2
                        nc.vector.tensor_copy(
                            v_aug[:, st0:st0 + 2, :, 0:HD],
                            psv.rearrange("s (t h c) -> s t h c", t=2, h=HL),
                        )
                    return emit

                def a_items(xts, half):
                    items = []
                    for ft in range(4):
                        for nt in range(2):
                            items.append(make_qk_item(xts, half, ft, nt))
                    for stp in range(4):
                        items.append(make_v_item(xts, half, stp))
                    return items

                xts0 = load_xts(0)
                for it in a_items(xts0, 0):
                    it()
                xts1 = load_xts(1)
                for it in a_items(xts1, 1):
                    it()
                filler = []

                if "B" not in stages:
                    # sink so DCE keeps stage A
                    dma.dma_start(
                        out=out[0:128, 0:1024],
                        in_=qkT[:, 0, 0:1024],
                    )
                    return

                # ======== stage B+C: per query macro-block ========
                # C work is drip-fed into B's matmul stream as PE filler.

                def make_c_item(st):
                    def emit():
                        ob = obp.tile([128, 1024], BF16)
                        for nt in range(2):
                            ps = ps_sc.tile([128, W], F32, tag="sc")
                            for dt_ in range(FV // 128):
                                nc.tensor.matmul(
                                    ps[:, 0:512],
                                    valuesT[:, dt_, st * 128:(st + 1) * 128],
                                    wo_sb[:, dt_, nt * 512:(nt + 1) * 512],
                                    start=(dt_ == 0),
                                    stop=(dt_ == FV // 128 - 1),
                                )
                            ob_copies[nt](
                                ob[:, nt * 512:(nt + 1) * 512], ps[:, 0:512]
                            )
                        dma.dma_start(
                            out=out[st * 128:(st + 1) * 128, :], in_=ob
                        )
                    return emit

                for qmb in range(N_QMB):
                    if qmb == 2:
                        while filler:
                            filler.pop(0)()
                    q0 = qmb * QMB
                    nkb = 4 * qmb + 4
                    nblk = nkb // pairw
                    for w0 in range(0, HL, wave):
                        whs = list(range(w0, w0 + wave))
                        avs = {
                            h_: ps_av.tile([65, QMB], F32, tag="av",
                                           name=f"av{h_}")
                            for h_ in whs
                        }
                        avq = []

                        def emit_av(item):
                            h, mms = item
                            for kb, col0, avw, ex_t in mms:
                                nc.tensor.matmul(
                                    avs[h][0:65, col0:col0 + avw],
                                    v_aug[:, kb, h, :],
                                    ex_t,
                                    start=(kb == 0),
                                    stop=(kb == nkb - 1),
                                )

                        for blk in range(nblk):
                            kb0 = blk * pairw
                            diag = kb0 + pairw - 1 >= 4 * qmb
                            scs = {}
                            # row-packed: both heads' score MMs emitted
                            # back-to-back; lhsT base partitions 0/64 ->
                            # concurrent row-group execution on the PE.
                            for h in whs:
                                scs[h] = ps_sc.tile(
                                    [128, W], F32, tag="sc",
                                    name=f"sc{h}"
                                )
                            for sub in range(pairw):
                                kb = kb0 + sub
                                j = kb - 4 * qmb
                                col0 = 128 * j if j >= 0 else 0
                                cb = sub * 512 + col0
                                scw = 512 - col0
                                for h in whs:
                                    tk = 2 * (h // 2)
                                    pk = 64 * (h % 2)
                                    nc.tensor.matmul(
                                        scs[h][:, cb:cb + scw],
                                        qkT[pk:pk + 64, tk,
                                            kb * KB:(kb + 1) * KB],
                                        qkT[pk:pk + 64, tk + 1,
                                            q0 + col0:q0 + col0 + scw],
                                        start=True,
                                        stop=True,
                                        skip_group_check=True,
                                    )
                            for h in whs:
                                sc = scs[h]
                                ex = expp.tile([128, W], bdt)
                                # exp: one full-tile act when the block's
                                # first sub starts at col 0 (fewer act
                                # instructions; garbage in dead regions is
                                # either masked below or never read by av),
                                # else per-sub trapezoid acts. Diagonal
                                # 128-blocks get the causal triangle zeroed
                                # post-exp (bf16 SBUF -> 4x DVE mode).
                                j0 = kb0 - 4 * qmb
                                if j0 <= 0:
                                    nc.scalar.activation(
                                        out=ex,
                                        in_=sc,
                                        func=(mybir
                                              .ActivationFunctionType.Exp),
                                        scale=SCALE,
                                    )
                                else:
                                    for sub in range(pairw):
                                        col0 = 128 * (j0 + sub)
                                        cb = sub * 512 + col0
                                        nc.scalar.activation(
                                            out=ex[:, cb:sub * 512 + 512],
                                            in_=sc[:, cb:sub * 512 + 512],
                                            func=(mybir
                                                  .ActivationFunctionType
                                                  .Exp),
                                            scale=SCALE,
                                        )
                                for sub in range(pairw):
                                    j = kb0 + sub - 4 * qmb
                                    if j >= 0:
                                        cb = sub * 512 + 128 * j
                                        mask_mul(
                                            ex[:, cb:cb + 128],
                                            ex[:, cb:cb + 128],
                                            tri01,
                                        )
                                mms = []
                                for sub in range(pairw):
                                    kb = kb0 + sub
                                    j = kb - 4 * qmb
                                    col0 = 128 * j if j >= 0 else 0
                                    avw = QMB - col0
                                    mms.append((
                                        kb, col0, avw,
                                        ex[:, sub * 512 + col0:
                                            sub * 512 + col0 + avw],
                                    ))
                                avq.append((h, mms))
                            if fill_first and filler:
                                filler.pop(0)()
                            while len(avq) > wave * lag:
                                emit_av(avq.pop(0))
                            if not fill_first and filler:
                                filler.pop(0)()
                        while avq:
                            emit_av(avq.pop(0))

                        # normalize: values = av[0:64] / av[64]
                        for h in whs:
                            av = avs[h]
                            rec = small.tile([1, QMB], F32R, tag="rec")
                            with nc.allow_low_precision(
                                reason="softmax denom feeds bf16 matmul"
                            ):
                                nc.vector.reciprocal(rec, av[64:65, :])
                            rb = small.tile([64, QMB], F32R, tag="rb")
                            nc.gpsimd.partition_broadcast(rb, rec)
                            dt_ = h // 2
                            pr = 64 * (h % 2)
                            nc.vector.tensor_mul(
                                valuesT[pr:pr + 64, dt_, q0:q0 + QMB],
                                av[0:64, :],
                                rb,
                            )
                    # ---- queue stage C for this qmb ----
                    if "C" not in stages:
                        dma.dma_start(
                            out=out[qmb * 128:(qmb + 1) * 128, 0:512],
                            in_=valuesT[:, 0, qmb * 512:qmb * 512 + 512],
                        )
                        continue
                    for sti in range(QMB // 128):
                        filler.append(make_c_item(qmb * 4 + sti))
                while filler:
                    filler.pop(0)()

            if repeat == 1:
                body(0)
            else:
                with tc.For_i(
                    0, repeat, 1,
                    hint_engines=(mybir.EngineType.PE,),
                    staggered_reset=staggered,
                ) as it:
                    body(it)
    nc.compile()
    return nc


def make_in_maps(x, W_qkv, b_qkv, W_out, b_out):
    """Host-side sharding: per-core input dict."""
    x = np.asarray(x, dtype=np.float32)
    W_qkv = np.asarray(W_qkv, dtype=np.float32)
    b_qkv = np.asarray(b_qkv, dtype=np.float32)
    W_out = np.asarray(W_out, dtype=np.float32)
    bf = ml_dtypes.bfloat16
    in_maps = []
    xT_by_b = [np.ascontiguousarray(x[b_].T).astype(bf) for b_ in range(B)]
    for c in range(N_CORES):
        b_ = c // 4
        g = c % 4
        heads = [4 * g + i for i in range(HL)]
        # feature order: K(h0),K(h1),Q(h0),Q(h1),K(h2),K(h3),Q(h2),Q(h3)
        qk_cols = []
        for pair in range(2):
            h0, h1 = heads[2 * pair], heads[2 * pair + 1]
            for h_ in (h0, h1):
                base = h_ * 3 * HD + 1 * HD  # K
                qk_cols.extend(range(base, base + HD))
            for h_ in (h0, h1):
                base = h_ * 3 * HD + 0 * HD  # Q
                qk_cols.extend(range(base, base + HD))
        v_cols = []
        for h_ in heads:
            base = h_ * 3 * HD + 2 * HD  # V
            v_cols.extend(range(base, base + HD))
        qk_cols = np.array(qk_cols)
        v_cols = np.array(v_cols)
        in_maps.append({
            "xT": xT_by_b[b_],
            "wqk": np.ascontiguousarray(W_qkv[:, qk_cols]).astype(bf),
            "wv": np.ascontiguousarray(W_qkv[:, v_cols]).astype(bf),
            "wo": np.ascontiguousarray(W_out[g * FV:(g + 1) * FV, :]).astype(bf),
            "bqk": np.ascontiguousarray(b_qkv[qk_cols]),
            "bv": np.ascontiguousarray(b_qkv[v_cols]),
        })
    return in_maps


_NC_CACHE = {}

# dev-loop hook: harness runs with this env unset -> compiled defaults
_ENV_KW = json.loads(os.environ.get("KERNEL_KW", "{}"))


def get_nc(repeat: int = 1):
    key = (repeat, tuple(sorted(_ENV_KW.items())))
    if key not in _NC_CACHE:
        _NC_CACHE[key] = build_kernel(repeat, **_ENV_KW)
    return _NC_CACHE[key]


def kernel(x, W_qkv, b_qkv, W_out, b_out):
    in_maps = make_in_maps(x, W_qkv, b_qkv, W_out, b_out)
    nc = get_nc(1)
    res = run_bass_kernel_spmd(nc, in_maps, list(range(N_CORES)))
    b_out = np.asarray(b_out, dtype=np.float32)
    out = np.zeros((B, S, D), dtype=np.float32)
    for b_ in range(B):
        acc = np.zeros((S, D), dtype=np.float32)
        for g in range(4):
            acc += np.asarray(res.results[4 * b_ + g]["out"]).astype(np.float32)
        out[b_] = acc + b_out[None, :]
    return out


# revision 19
# speedup vs baseline: 1.0609x; 1.0609x over previous
"""Multi-head causal attention (B=2, S=2048, D=1024, H=16) on 8 trn2 cores.

Sharding: core c -> (batch b = c//4, head-group g = c%4, 4 heads each).
Data-parallel over B, tensor-parallel over heads. Each core computes a
partial output projection [S, D] in bf16; the host sums the 4 partials
per batch (in f32) and adds b_out.

Device kernel per core (matmuls in bf16, accumulation in f32 PSUM):
  A) qkT[f=512, s=2048] = (x @ Wqk)^T and v[s, f=256] = x @ Wv (+biases,
     folded in as K=1 rank-1 f32r matmuls into PSUM).
     qkT feature layout: [K(h0)|K(h1)] [Q(h0)|Q(h1)] [K(h2)|K(h3)] [Q(h2)|Q(h3)]
     PSUM->SBUF(bf16) moves: qkT on the Act engine, v on DVE.
  B) per head h, per 512-wide query block qmb: causal flash attention in
     the scores-TRANSPOSED layout: sT[k,q] = K @ Q^T so that attn@V is
     lhsT=v_blk[s,hd+1] (ones col appended -> softmax denominators in
     row 64 of PSUM), rhs=expT[k,q]. No on-chip transposes anywhere.
     Causal mask applied POST-exp: DVE multiplies the diagonal 128x128
     blocks of ex (bf16, SBUF -> 4x DVE mode) by a 0/1 triangle. This
     keeps the PE free of mask matmuls/Ldweights.
     The sc->exp->av chains are software-pipelined (av emission lags by
     `lag` links) and rotate through a deep PSUM pool so cross-engine
     semaphore wake latency is hidden.
  C) out_partial[s, 1024] = values^T.T @ W_out; PSUM->SBUF copies on the
     (otherwise idle) Pool engine, DMA'd to DRAM as bf16.
"""
import json
import math
import os

import numpy as np
import ml_dtypes

import concourse.bass as bass
import concourse.mybir as mybir
import concourse.tile as tile
from concourse import bacc
from concourse.bass_utils import run_bass_kernel_spmd

N_CORES = 8
B, S, D = 2, 2048, 1024
H = 16                    # total heads
HL = 4                    # heads per core
HD = 64                   # head dim
FQK = 2 * HL * HD         # 512 local q+k features
FV = HL * HD              # 256 local v features
SCALE = 1.0 / math.sqrt(HD)

QMB = 512                 # query macro-block
KB = 128                  # key block
N_QMB = S // QMB          # 4
N_KB = S // KB            # 16

F32 = mybir.dt.float32
F32R = mybir.dt.float32r
BF16 = mybir.dt.bfloat16


def build_kernel(repeat: int = 1, stages: str = "ABC",
                 bmode: str = "full", pairw: int = 2, wave: int = 2,
                 sc_bufs: int = 3, av_bufs: int = 2, exp_bufs: int = 8,
                 lag: int = 2,
                 fill_first: bool = False,
                 qk_copy_eng: str = "scalar",
                 ob_eng: str = "vector",
                 mask_eng: str = "vector",
                 bdt_name: str = "bf16",
                 staggered: bool = False):
    assert sc_bufs * pairw + av_bufs <= 8
    W = 512 * pairw
    # internal dtype for qkT / v_aug / ex / valuesT (and wo to match C MMs):
    # bf16 halves SBUF + full-rate short MMs, but legalization splits every
    # 2-byte matmul into Ldweights+Matmult (extra PE.SEQ issue slots);
    # f32r self-loads (no Ldweights) at 1 cyc/row for >=256-wide MMs.
    bdt = BF16 if bdt_name == "bf16" else F32R
    nc = bacc.Bacc(
        "TRN2", target_bir_lowering=False, debug=False, num_devices=N_CORES
    )
    xT = nc.dram_tensor("xT", [D, S], BF16, kind="ExternalInput")
    wqk = nc.dram_tensor("wqk", [D, FQK], BF16, kind="ExternalInput")
    wv = nc.dram_tensor("wv", [D, FV], BF16, kind="ExternalInput")
    wo = nc.dram_tensor("wo", [FV, D], bdt, kind="ExternalInput")
    bqk = nc.dram_tensor("bqk", [FQK], F32R, kind="ExternalInput")
    bv = nc.dram_tensor("bv", [FV], F32R, kind="ExternalInput")
    out = nc.dram_tensor("out", [S, D], BF16, kind="ExternalOutput")

    KT = D // 128  # 8 contraction tiles over D

    with tile.TileContext(nc) as tc:
        dma = nc.sync  # HWDGE: spreads transfers over HW queues
        with (
            tc.tile_pool(name="const", bufs=1) as const,
            tc.tile_pool(name="xt", bufs=1) as xtp,
            tc.tile_pool(name="big", bufs=1) as big,
            tc.tile_pool(name="exp", bufs=exp_bufs) as expp,
            tc.tile_pool(name="small", bufs=4) as small,
            tc.tile_pool(name="ob", bufs=3) as obp,
            tc.tile_pool(name="ps_sc", bufs=sc_bufs, space="PSUM") as ps_sc,
            tc.tile_pool(name="ps_av", bufs=av_bufs, space="PSUM") as ps_av,
        ):
            # ---- constants ----
            wqk_sb = const.tile([128, KT, FQK], BF16)
            wv_sb = const.tile([128, KT, FV], BF16)
            wo_sb = const.tile([128, FV // 128, D], bdt)
            dma.dma_start(
                out=wqk_sb, in_=wqk.rearrange("(kt p) f -> p kt f", p=128)
            )
            dma.dma_start(
                out=wv_sb, in_=wv.rearrange("(kt p) f -> p kt f", p=128)
            )
            dma.dma_start(
                out=wo_sb, in_=wo.rearrange("(dt p) f -> p dt f", p=128)
            )
            # qk bias laid out per-partition [feature%128, ft] so the Act
            # engine can fold it into the PSUM->SBUF copy (Identity+bias);
            # v bias stays a rank-1 matmul (its features are on the free dim).
            bqk_pb = const.tile([128, FQK // 128], F32R)
            dma.dma_start(
                out=bqk_pb, in_=bqk.rearrange("(ft p) -> p ft", p=128)
            )
            # v bias broadcast to all partitions (twice along free dim) so
            # the DVE PSUM->SBUF move of v becomes a fused add.
            bv_sb = const.tile([1, 2, FV], F32R)
            for rep_ in range(2):
                dma.dma_start(
                    out=bv_sb[:, rep_, :],
                    in_=bv.rearrange("(o f) -> o f", o=1),
                )
            bv_pb = const.tile([128, 2, FV], F32R)
            nc.gpsimd.partition_broadcast(
                bv_pb.rearrange("p t f -> p (t f)"),
                bv_sb.rearrange("o t f -> o (t f)"),
            )
            # multiplicative causal mask for the diagonal 128x128 block:
            # tri01[k, q] = 1 if k <= q else 0  (bf16, for 4x DVE mode)
            tri_f32 = const.tile([128, 128], F32)
            nc.gpsimd.memset(tri_f32, 1.0)
            nc.gpsimd.affine_select(
                out=tri_f32,
                in_=tri_f32,
                compare_op=mybir.AluOpType.is_ge,
                fill=0.0,
                base=0,
                pattern=[[1, 128]],
                channel_multiplier=-1,
            )
            tri01 = const.tile([128, 128], bdt)
            nc.vector.tensor_copy(tri01, tri_f32)

            # ---- persistent intermediates ----
            qkT = big.tile([128, 4, S], bdt)            # 4 f-tiles x S
            v_aug = big.tile([128, N_KB, HL, HD + 1], bdt)
            valuesT = big.tile([128, FV // 128, S], bdt)
            vone_f32 = const.tile([128, N_KB * HL], F32)
            nc.vector.memset(vone_f32, 1.0)
            nc.vector.tensor_copy(
                v_aug[:, :, :, HD:HD + 1],
                vone_f32.rearrange("p (kb h o) -> p kb h o", h=HL, o=1),
            )

            # NOTE: GpSimd (Pool) cannot access PSUM on TRN2 (BIR verifier
            # rejects it), so PSUM->SBUF moves must use DVE or Act.
            if ob_eng == "vector":
                ob_copies = (nc.vector.tensor_copy, nc.vector.tensor_copy)
            elif ob_eng == "scalar":
                ob_copies = (nc.scalar.copy, nc.scalar.copy)
            else:  # "split": one half DVE, one half Act
                ob_copies = (nc.vector.tensor_copy, nc.scalar.copy)
            mask_mul = (nc.vector.tensor_mul if mask_eng == "vector"
                        else nc.gpsimd.tensor_mul)

            def body(_it):
                # ======== stage A: qkT and v_aug, in two column halves ====
                def load_xts(half):
                    s0 = half * (S // 2)
                    xts = []
                    for kt in range(KT):
                        xt_t = xtp.tile([128, S // 2], BF16, tag=f"xt{kt}")
                        dma.dma_start(
                            out=xt_t,
                            in_=xT[kt * 128:(kt + 1) * 128, s0:s0 + S // 2],
                        )
                        xts.append(xt_t)
                    return xts

                def make_qk_item(xts, half, ft, nt):
                    s0 = half * (S // 2)

                    def emit():
                        c0 = nt * 512
                        ps = ps_sc.tile([128, W], F32, tag="sc")
                        for kt in range(KT):
                            nc.tensor.matmul(
                                ps[:, 0:512],
                                wqk_sb[:, kt, ft * 128:(ft + 1) * 128],
                                xts[kt][:, c0:c0 + 512],
                                start=(kt == 0),
                                stop=(kt == KT - 1),
                            )
                        if qk_copy_eng == "scalar":
                            nc.scalar.activation(
                                out=qkT[:, ft, s0 + c0:s0 + c0 + 512],
                                in_=ps[:, 0:512],
                                func=mybir.ActivationFunctionType.Identity,
                                bias=bqk_pb[:, ft:ft + 1],
                            )
                        else:
                            nc.vector.tensor_scalar_add(
                                qkT[:, ft, s0 + c0:s0 + c0 + 512],
                                ps[:, 0:512],
                                bqk_pb.bitcast(F32)[:, ft:ft + 1],
                            )
                    return emit

                def make_v_item(xts, half, stp):
                    def emit():
                        psv = ps_sc.tile([128, 512], F32, tag="sc")
                        for sub in range(2):
                            sti = stp * 2 + sub
                            c0 = sub * FV
                            for kt in range(KT):
                                nc.tensor.matmul(
                                    psv[:, c0:c0 + FV],
                                    xts[kt][:, sti * 128:(sti + 1) * 128],
                                    wv_sb[:, kt, :],
                                    start=(kt == 0),
                                    stop=False,
                                )
                            nc.tensor.matmul(
                                psv[:, c0:c0 + FV],
                                ones_row[0:1, 0:128],
                                bv_sb,
                                start=False,
                                stop=True,
                            )
                        st0 = half * 8 + stp * 2
                        nc.vector.tensor_copy(
                            v_aug[:, st0:st0 + 2, :, 0:HD],
                            psv.rearrange("s (t h c) -> s t h c", t=2, h=HL),
                        )
                    return emit

                def a_items(xts, half):
                    items = []
                    for ft in range(4):
                        for nt in range(2):
                            items.append(make_qk_item(xts, half, ft, nt))
                    for stp in range(4):
                        items.append(make_v_item(xts, half, stp))
                    return items

                xts0 = load_xts(0)
                for it in a_items(xts0, 0):
                    it()
                xts1 = load_xts(1)
                for it in a_items(xts1, 1):
                    it()
                filler = []

                if "B" not in stages:
                    # sink so DCE keeps stage A
                    dma.dma_start(
                        out=out[0:128, 0:1024],
                        in_=qkT[:, 0, 0:1024],
                    )
                    return

                # ======== stage B+C: per query macro-block ========
                # C work is drip-fed into B's matmul stream as PE filler.

                def make_c_item(st):
                    def emit():
                        ob = obp.tile([128, 1024], BF16)
                        for nt in range(2):
                            ps = ps_sc.tile([128, W], F32, tag="sc")
                            for dt_ in range(FV // 128):
                                nc.tensor.matmul(
                                    ps[:, 0:512],
                                    valuesT[:, dt_, st * 128:(st + 1) * 128],
                                    wo_sb[:, dt_, nt * 512:(nt + 1) * 512],
                                    start=(dt_ == 0),
                                    stop=(dt_ == FV // 128 - 1),
                                )
                            ob_copies[nt](
                                ob[:, nt * 512:(nt + 1) * 512], ps[:, 0:512]
                            )
                        dma.dma_start(
                            out=out[st * 128:(st + 1) * 128, :], in_=ob
                        )
                    return emit

                for qmb in range(N_QMB):
                    if qmb == 2:
                        while filler:
                            filler.pop(0)()
                    q0 = qmb * QMB
                    nkb = 4 * qmb + 4
                    nblk = nkb // pairw
                    for w0 in range(0, HL, wave):
                        whs = list(range(w0, w0 + wave))
                        avs = {
                            h_: ps_av.tile([65, QMB], F32, tag="av",
                                           name=f"av{h_}")
                            for h_ in whs
                        }
                        avq = []

                        def emit_av(item):
                            h, mms = item
                            for kb, col0, avw, ex_t in mms:
                                nc.tensor.matmul(
                                    avs[h][0:65, col0:col0 + avw],
                                    v_aug[:, kb, h, :],
                                    ex_t,
                                    start=(kb == 0),
                                    stop=(kb == nkb - 1),
                                )

                        for blk in range(nblk):
                            kb0 = blk * pairw
                            diag = kb0 + pairw - 1 >= 4 * qmb
                            scs = {}
                            # row-packed: both heads' score MMs emitted
                            # back-to-back; lhsT base partitions 0/64 ->
                            # concurrent row-group execution on the PE.
                            for h in whs:
                                scs[h] = ps_sc.tile(
                                    [128, W], F32, tag="sc",
                                    name=f"sc{h}"
                                )
                            for sub in range(pairw):
                                kb = kb0 + sub
                                j = kb - 4 * qmb
                                col0 = 128 * j if j >= 0 else 0
                                cb = sub * 512 + col0
                                scw = 512 - col0
                                for h in whs:
                                    tk = 2 * (h // 2)
                                    pk = 64 * (h % 2)
                                    nc.tensor.matmul(
                                        scs[h][:, cb:cb + scw],
                                        qkT[pk:pk + 64, tk,
                                            kb * KB:(kb + 1) * KB],
                                        qkT[pk:pk + 64, tk + 1,
                                            q0 + col0:q0 + col0 + scw],
                                        start=True,
                                        stop=True,
                                        skip_group_check=True,
                                    )
                            for h in whs:
                                sc = scs[h]
                                ex = expp.tile([128, W], bdt)
                                # exp: one full-tile act when the block's
                                # first sub starts at col 0 (fewer act
                                # instructions; garbage in dead regions is
                                # either masked below or never read by av),
                                # else per-sub trapezoid acts. Diagonal
                                # 128-blocks get the causal triangle zeroed
                                # post-exp (bf16 SBUF -> 4x DVE mode).
                                j0 = kb0 - 4 * qmb
                                if j0 <= 0:
                                    nc.scalar.activation(
                                        out=ex,
                                        in_=sc,
                                        func=(mybir
                                              .ActivationFunctionType.Exp),
                                        scale=SCALE,
                                    )
                                else:
                                    for sub in range(pairw):
                                        col0 = 128 * (j0 + sub)
                                        cb = sub * 512 + col0
                                        nc.scalar.activation(
                                            out=ex[:, cb:sub * 512 + 512],
                                            in_=sc[:, cb:sub * 512 + 512],
                                            func=(mybir
                                                  .ActivationFunctionType
                                                  .Exp),
                                            scale=SCALE,
                                        )
                                for sub in range(pairw):
                                    j = kb0 + sub - 4 * qmb
                                    if j >= 0:
                                        cb = sub * 512 + 128 * j
                                        mask_mul(
                                            ex[:, cb:cb + 128],
                                            ex[:, cb:cb + 128],
                                            tri01,
                                        )
                                mms = []
                                for sub in range(pairw):
                                    kb = kb0 + sub
                                    j = kb - 4 * qmb
                                    col0 = 128 * j if j >= 0 else 0
                                    avw = QMB - col0
                                    mms.append((
                                        kb, col0, avw,
                                        ex[:, sub * 512 + col0:
                                            sub * 512 + col0 + avw],
                                    ))
                                avq.append((h, mms))
                            if fill_first and filler:
                                filler.pop(0)()
                            while len(avq) > wave * lag:
                                emit_av(avq.pop(0))
                            if not fill_first and filler:
                                filler.pop(0)()
                        while avq:
                            emit_av(avq.pop(0))

                        # normalize: values = av[0:64] / av[64]
                        for h in whs:
                            av = avs[h]
                            rec = small.tile([1, QMB], F32R, tag="rec")
                            with nc.allow_low_precision(
                                reason="softmax denom feeds bf16 matmul"
                            ):
                                nc.vector.reciprocal(rec, av[64:65, :])
                            rb = small.tile([64, QMB], F32R, tag="rb")
                            nc.gpsimd.partition_broadcast(rb, rec)
                            dt_ = h // 2
                            pr = 64 * (h % 2)
                            nc.vector.tensor_mul(
                                valuesT[pr:pr + 64, dt_, q0:q0 + QMB],
                                av[0:64, :],
                                rb,
                            )
                    # ---- queue stage C for this qmb ----
                    if "C" not in stages:
                        dma.dma_start(
                            out=out[qmb * 128:(qmb + 1) * 128, 0:512],
                            in_=valuesT[:, 0, qmb * 512:qmb * 512 + 512],
                        )
                        continue
                    for sti in range(QMB // 128):
                        filler.append(make_c_item(qmb * 4 + sti))
                while filler:
                    filler.pop(0)()

            if repeat == 1:
                body(0)
            else:
                with tc.For_i(
                    0, repeat, 1,
                    hint_engines=(mybir.EngineType.PE,),
                    staggered_reset=staggered,
                ) as it:
                    body(it)
    nc.compile()
    return nc


def make_in_maps(x, W_qkv, b_qkv, W_out, b_out):
    """Host-side sharding: per-core input dict."""
    x = np.asarray(x, dtype=np.float32)
    W_qkv = np.asarray(W_qkv, dtype=np.float32)
    b_qkv = np.asarray(b_qkv, dtype=np.float32)
    W_out = np.asarray(W_out, dtype=np.float32)
    bf = ml_dtypes.bfloat16
    in_maps = []
    xT_by_b = [np.ascontiguousarray(x[b_].T).astype(bf) for b_ in range(B)]
    for c in range(N_CORES):
        b_ = c // 4
        g = c % 4
        heads = [4 * g + i for i in range(HL)]
        # feature order: K(h0),K(h1),Q(h0),Q(h1),K(h2),K(h3),Q(h2),Q(h3)
        qk_cols = []
        for pair in range(2):
            h0, h1 = heads[2 * pair], heads[2 * pair + 1]
            for h_ in (h0, h1):
                base = h_ * 3 * HD + 1 * HD  # K
                qk_cols.extend(range(base, base + HD))
            for h_ in (h0, h1):
                base = h_ * 3 * HD + 0 * HD  # Q
                qk_cols.extend(range(base, base + HD))
        v_cols = []
        for h_ in heads:
            base = h_ * 3 * HD + 2 * HD  # V
            v_cols.extend(range(base, base + HD))
        qk_cols = np.array(qk_cols)
        v_cols = np.array(v_cols)
        in_maps.append({
            "xT": xT_by_b[b_],
            "wqk": np.ascontiguousarray(W_qkv[:, qk_cols]).astype(bf),
            "wv": np.ascontiguousarray(W_qkv[:, v_cols]).astype(bf),
            "wo": np.ascontiguousarray(W_out[g * FV:(g + 1) * FV, :]).astype(bf),
            "bqk": np.ascontiguousarray(b_qkv[qk_cols]),
            "bv": np.ascontiguousarray(b_qkv[v_cols]),
        })
    return in_maps


_NC_CACHE = {}

# dev-loop hook: harness runs with this env unset -> compiled defaults
_ENV_KW = json.loads(os.environ.get("KERNEL_KW", "{}"))


def get_nc(repeat: int = 1):
    key = (repeat, tuple(sorted(_ENV_KW.items())))
    if key not in _NC_CACHE:
        _NC_CACHE[key] = build_kernel(repeat, **_ENV_KW)
    return _NC_CACHE[key]


def kernel(x, W_qkv, b_qkv, W_out, b_out):
    in_maps = make_in_maps(x, W_qkv, b_qkv, W_out, b_out)
    nc = get_nc(1)
    res = run_bass_kernel_spmd(nc, in_maps, list(range(N_CORES)))
    b_out = np.asarray(b_out, dtype=np.float32)
    out = np.zeros((B, S, D), dtype=np.float32)
    for b_ in range(B):
        acc = np.zeros((S, D), dtype=np.float32)
        for g in range(4):
            acc += np.asarray(res.results[4 * b_ + g]["out"]).astype(np.float32)
        out[b_] = acc + b_out[None, :]
    return out
